# revision 7
# baseline (speedup 1.0000x reference)
"""GAT (graph attention) kernel for 8 Trainium2 NeuronCores.

Strategy — fused edge-replicated dataflow (no device-side gather):
  * Core k owns dst nodes [k*npc, (k+1)*npc).  Host appends self-loops and
    buckets edges by dst chunk of 128, padding each chunk to a multiple of
    128 edge slots with uniform tile counts across cores so ONE SPMD
    program serves all 8 cores (per the vertex-cut sharding hint).
  * The host ships source-node FEATURES replicated per edge slot
    (xeT[:, slot] = x[:, src(slot)] — the halo-exchange/layout step of the
    1D graph partitioning, done once on the host), so the device never
    performs a data-dependent gather: a previous revision gathered 768B
    h-rows per edge with SWDGE dma_gather, which costs ~8.4 ns/edge of
    Q7 descriptor-generation time (~2.1 ms/core) regardless of row size.
  * Device, per edge tile of 128 slots: h|a_src = xe @ [W|w_src] straight
    into PSUM (f16 matmuls), per-edge a_dst accumulated INTO the same PSUM
    columns by a one-hot matmul (host-precomputed fp8 one-hot masks),
    leaky-relu+exp on the scalar engine, alpha*h on DVE, and the one-hot
    scatter matmul accumulates [out | denom] per dst chunk in PSUM.
    Messages live SBUF/PSUM-only — h never round-trips through DRAM.
  * Chunk tail: softmax division, bias, relu, L2-normalize, store.
    exp() skips the segment-max shift: logits are O(10) so exp stays in
    range, and softmax is shift-invariant, so results are identical.
  * a_dst per dst node comes from a tiny side matmul x_own @ w_dst per
    chunk (w_src/w_dst fold att_src/att_dst into W; host parameter fusion).
"""

import os
import sys

sys.path.insert(0, "/opt/trn_rl_repo")

import numpy as np

HEADS = 4
OUT_CH = 64
NEG_SLOPE = 0.2
P = 128


# --------------------------------------------------------------------------
# host-side preprocessing (sharding + layout only, plus parameter fusion)
# --------------------------------------------------------------------------
def _preprocess(x, edge_index, W, att_src, att_dst, bias, n_cores):
    import ml_dtypes

    mask_np = (
        np.float16
        if os.environ.get("GAT_MASK_DTYPE", "f8") == "f16"
        else ml_dtypes.float8_e4m3
    )

    x = np.asarray(x, np.float32)
    N, IN = x.shape
    assert N % n_cores == 0
    npc = N // n_cores
    chunks = (npc + P - 1) // P

    src = np.concatenate(
        [np.asarray(edge_index[0], np.int64), np.arange(N, dtype=np.int64)]
    )
    dst = np.concatenate(
        [np.asarray(edge_index[1], np.int64), np.arange(N, dtype=np.int64)]
    )

    core = dst // npc
    rem = dst - core * npc
    chunk = rem // P
    dstl = (rem - chunk * P).astype(np.int16)

    # per-core edges sorted by dst chunk
    per_core = []
    for k in range(n_cores):
        sel = np.nonzero(core == k)[0]
        order = np.argsort(chunk[sel], kind="stable")
        sel = sel[order]
        counts = np.bincount(chunk[sel], minlength=chunks)
        per_core.append((src[sel], dstl[sel], counts))

    all_counts = np.stack([pc[2] for pc in per_core])  # [cores, chunks]
    Tch = np.maximum(1, -(-all_counts.max(axis=0) // P))  # [chunks]
    total_slots = int(P * Tch.sum())
    TT = int(Tch.sum())
    tile_off = np.zeros(chunks + 1, np.int64)
    np.cumsum(Tch, out=tile_off[1:])

    xT16 = np.ascontiguousarray(x.T).astype(np.float16)  # [IN, N]

    # parameter-only fusion: a_src = h @ att_src == x @ w_src
    W4 = np.asarray(W, np.float32).reshape(IN, HEADS, OUT_CH)
    w_src = np.einsum("ihc,hc->ih", W4, np.asarray(att_src, np.float32))
    w_dst = np.einsum("ihc,hc->ih", W4, np.asarray(att_dst, np.float32))
    Wsrc = np.ascontiguousarray(
        np.concatenate([np.asarray(W, np.float32), w_src], axis=1)
    ).astype(np.float16)  # [IN, 260]
    wdst16 = np.ascontiguousarray(w_dst).astype(np.float16)  # [IN, 4]

    d_iota = np.arange(P, dtype=np.int16)
    in_maps = []
    for k in range(n_cores):
        src_k, dstl_k, counts = per_core[k]
        src_slot = np.zeros(total_slots, np.int64)
        dstl_slot = np.full(total_slots, -1, np.int16)
        for c in range(chunks):
            o = int(tile_off[c]) * P
            s0 = int(counts[:c].sum())
            n = int(counts[c])
            src_slot[o : o + n] = src_k[s0 : s0 + n]
            dstl_slot[o : o + n] = dstl_k[s0 : s0 + n]

        dstl_r = dstl_slot.reshape(TT, P)
        # m4[p, t, d] = (dstl of slot (t,p)) == d  (edge-major one-hot)
        m4 = (dstl_r.T[:, :, None] == d_iota[None, None, :]).astype(mask_np)
        # mT[d, t, e] = (dstl of slot (t,e)) == d  (dst-major one-hot)
        mT = (d_iota[:, None, None] == dstl_r[None, :, :]).astype(mask_np)

        in_maps.append(
            {
                "xeT": np.ascontiguousarray(xT16[:, src_slot]),
                "xoT": np.ascontiguousarray(
                    xT16[:, k * npc : (k + 1) * npc]
                ),
                "m4": np.ascontiguousarray(m4.reshape(P, TT * P)),
                "mT": np.ascontiguousarray(mT.reshape(P, TT * P)),
                "Wsrc": Wsrc,
                "wdst": wdst16,
                "bias": np.asarray(bias, np.float32),
            }
        )

    meta = dict(
        N=N, IN=IN, npc=npc, chunks=chunks, Tch=Tch, tile_off=tile_off, TT=TT
    )
    return meta, in_maps


# --------------------------------------------------------------------------
# device program (identical on every core)
# --------------------------------------------------------------------------
def _build_program(meta, n_cores, debug=False):
    import concourse.bacc as bacc
    import concourse.mybir as mybir
    import concourse.tile as tile

    f32 = mybir.dt.float32
    f16 = mybir.dt.float16
    i32 = mybir.dt.int32
    f8 = mybir.dt.float8e4
    mkdt = f16 if os.environ.get("GAT_MASK_DTYPE", "f8") == "f16" else f8
    # offload PSUM->SBUF logit copies and half the alpha*h multiplies to
    # the (otherwise idle) Pool engine; fallback: everything on DVE
    use_pool = os.environ.get("GAT_POOL", "1") == "1"
    # tail 1/sqrt via Quake seed + Newton on DVE (keeps the scalar engine
    # on one activation table set); fallback: exp(-0.5*ln(s)) on scalar
    rsqrt_newton = os.environ.get("GAT_RSQRT", "newton") == "newton"

    N, IN = meta["N"], meta["IN"]
    npc, chunks = meta["npc"], meta["chunks"]
    Tch, tile_off = meta["Tch"], meta["tile_off"]
    TT = meta["TT"]
    AUGS = IN + HEADS  # 260
    KB = IN // P  # contraction blocks (2)
    XB = 8  # edge tiles per xe load

    nc = bacc.Bacc(
        "TRN2", target_bir_lowering=False, debug=debug, num_devices=n_cores
    )

    def mm(out, lhsT, rhs, **kw):
        nc.tensor.matmul(out, lhsT, rhs, **kw)

    xeT_d = nc.dram_tensor("xeT", [IN, TT * P], f16, kind="ExternalInput")
    xoT_d = nc.dram_tensor("xoT", [IN, npc], f16, kind="ExternalInput")
    m4_d = nc.dram_tensor("m4", [P, TT * P], mkdt, kind="ExternalInput")
    mT_d = nc.dram_tensor("mT", [P, TT * P], mkdt, kind="ExternalInput")
    Wsrc_d = nc.dram_tensor("Wsrc", [IN, AUGS], f16, kind="ExternalInput")
    wdst_d = nc.dram_tensor("wdst", [IN, HEADS], f16, kind="ExternalInput")
    bias_d = nc.dram_tensor("bias", [IN], f32, kind="ExternalInput")
    out_d = nc.dram_tensor("out", [npc, IN], f32, kind="ExternalOutput")

    with tile.TileContext(nc) as tc:
        with tc.tile_pool(name="const", bufs=1) as cpool:
            ones_row = cpool.tile([1, P], f32)
            nc.vector.memset(ones_row[:], 1.0)

            bias_row = cpool.tile([1, IN], f32)
            nc.sync.dma_start(out=bias_row[:], in_=bias_d[None, :])
            bias_full = cpool.tile([P, HEADS, OUT_CH], f32)
            with tc.tile_pool(name="cpsum", bufs=1, space="PSUM") as cpsum:
                bias_psum = cpsum.tile([P, HEADS, OUT_CH], f32)
                nc.tensor.matmul(
                    bias_psum[:], ones_row[:], bias_row[:], start=True, stop=True
                )
                nc.vector.tensor_copy(bias_full[:], bias_psum[:])

            Wsrc_sb = cpool.tile([P, KB, AUGS], f16)
            wdst_sb = cpool.tile([P, KB, HEADS], f16)
            for k in range(KB):
                nc.sync.dma_start(
                    out=Wsrc_sb[:, k, :], in_=Wsrc_d[k * P : (k + 1) * P, :]
                )
                nc.sync.dma_start(
                    out=wdst_sb[:, k, :], in_=wdst_d[k * P : (k + 1) * P, :]
                )

            with (
                tc.tile_pool(name="xe", bufs=3) as xepool,
                tc.tile_pool(name="xo", bufs=2) as xopool,
                tc.tile_pool(name="mk", bufs=2) as mkpool,
                tc.tile_pool(name="adst", bufs=2) as adpool,
                tc.tile_pool(name="work", bufs=4) as wpool,
                tc.tile_pool(name="rhs", bufs=4) as rpool,
                tc.tile_pool(name="tail", bufs=2) as fpool,
                tc.tile_pool(name="hpsum", bufs=4, space="PSUM") as hpsum,
                tc.tile_pool(name="opsum", bufs=2, space="PSUM") as opsum,
                tc.tile_pool(name="apsum", bufs=2, space="PSUM") as apsum,
            ):
                for c in range(chunks):
                    Tc = int(Tch[c])
                    toff = int(tile_off[c])
                    pc = min(P, npc - c * P)

                    # a_dst of this chunk's own dst nodes: x_own @ w_dst
                    xo = xopool.tile([P, KB, P], f16, tag="xo")
                    for k in range(KB):
                        nc.scalar.dma_start(
                            out=xo[:, k, :pc],
                            in_=xoT_d[k * P : (k + 1) * P, c * P : c * P + pc],
                        )
                    adp = apsum.tile([P, HEADS], f32)
                    for k in range(KB):
                        mm(
                            adp[:pc, :],
                            xo[:, k, :pc],
                            wdst_sb[:, k, :],
                            start=(k == 0),
                            stop=(k == KB - 1),
                        )
                    adst_sb = adpool.tile([P, HEADS], f16, tag="adst")
                    nc.vector.tensor_copy(adst_sb[:pc, :], adp[:pc, :])

                    m4_sb = mkpool.tile([P, Tc, P], mkdt, tag="m4")
                    nc.sync.dma_start(
                        out=m4_sb[:],
                        in_=m4_d[:, toff * P : (toff + Tc) * P],
                    )
                    mT_sb = mkpool.tile([P, Tc, P], mkdt, tag="mT")
                    nc.sync.dma_start(
                        out=mT_sb[:],
                        in_=mT_d[:, toff * P : (toff + Tc) * P],
                    )

                    out_ps = opsum.tile([P, 4, 65], f32)
                    EB = 3  # edge tiles per batched lrelu/exp group
                    for g0 in range(0, Tc, EB):
                        nb = min(EB, Tc - g0)
                        el0 = wpool.tile([P, EB, HEADS], f32, tag="el0")
                        el16 = wpool.tile([P, EB, HEADS], f16, tag="el16")
                        brhs = rpool.tile([P, EB, 4, 65], f16, tag="brhs")
                        hps = []
                        for j in range(nb):
                            t = g0 + j
                            if t % XB == 0:
                                nxb = min(XB, Tc - t)
                                s0 = (toff + t) * P
                                xe = xepool.tile(
                                    [P, KB, XB * P], f16, tag="xe"
                                )
                                for k in range(KB):
                                    eng = (
                                        nc.sync
                                        if (t // XB) % 2 == 0
                                        else nc.scalar
                                    )
                                    eng.dma_start(
                                        out=xe[:, k, : nxb * P],
                                        in_=xeT_d[
                                            k * P : (k + 1) * P,
                                            s0 : s0 + nxb * P,
                                        ],
                                    )
                                xe_cur = xe
                            xs = (t % XB) * P
                            # h|a_src for this edge tile, in PSUM; per-edge
                            # a_dst accumulates into the a_src columns
                            hp = hpsum.tile([P, AUGS], f32)
                            for k in range(KB):
                                mm(
                                    hp[:],
                                    xe_cur[:, k, xs : xs + P],
                                    Wsrc_sb[:, k, :],
                                    start=(k == 0),
                                    stop=False,
                                )
                            mm(
                                hp[:, IN : IN + HEADS],
                                mT_sb[:, t, :],
                                adst_sb[:],
                                start=False,
                                stop=True,
                            )
                            ceng = nc.gpsimd if use_pool else nc.vector
                            ceng.tensor_copy(
                                el0[:, j, :], hp[:, IN : IN + HEADS]
                            )
                            hps.append((hp, t, j))
                        # batched leaky-relu (DVE) + exp (scalar)
                        nc.vector.scalar_tensor_tensor(
                            out=el16[:, :nb, :],
                            in0=el0[:, :nb, :],
                            scalar=NEG_SLOPE,
                            in1=el0[:, :nb, :],
                            op0=mybir.AluOpType.mult,
                            op1=mybir.AluOpType.max,
                        )
                        nc.scalar.activation(
                            brhs[:, :nb, :, 64],
                            el16[:, :nb, :],
                            mybir.ActivationFunctionType.Exp,
                        )
                        for hp, t, j in hps:
                            meng = (
                                nc.gpsimd if (use_pool and t % 2) else nc.vector
                            )
                            meng.tensor_tensor(
                                out=brhs[:, j, :, 0:64],
                                in0=hp[:, 0:IN].rearrange(
                                    "p (h c) -> p h c", h=HEADS
                                ),
                                in1=brhs[:, j, :, 64:65].to_broadcast(
                                    [P, HEADS, OUT_CH]
                                ),
                                op=mybir.AluOpType.mult,
                            )
                            mm(
                                out_ps[:],
                                m4_sb[:, t, :],
                                brhs[:, j],
                                start=(t == 0),
                                stop=(t == Tc - 1),
                            )

                    # chunk tail: softmax division, bias, relu, L2 norm
                    dn = fpool.tile([P, HEADS], f32, tag="dn")
                    nc.vector.tensor_scalar_max(dn[:], out_ps[:, :, 64], 1e-30)
                    rdn = fpool.tile([P, HEADS], f32, tag="rdn")
                    nc.vector.reciprocal(rdn[:], dn[:])
                    o1 = fpool.tile([P, HEADS, OUT_CH], f32, tag="o1")
                    nc.vector.tensor_tensor(
                        out=o1[:],
                        in0=out_ps[:, :, 0:64],
                        in1=rdn[:, :, None].to_broadcast([P, HEADS, OUT_CH]),
                        op=mybir.AluOpType.mult,
                    )
                    nc.vector.tensor_add(o1[:], o1[:], bias_full[:])
                    o2 = fpool.tile([P, HEADS, OUT_CH], f32, tag="o2")
                    nc.scalar.activation(
                        o2[:], o1[:], mybir.ActivationFunctionType.Relu
                    )
                    # s = sum(o2^2) via scalar-engine Square w/ accumulate
                    sq = fpool.tile([P, HEADS, OUT_CH], f16, tag="sq")
                    s = fpool.tile([P, 1], f32, tag="s")
                    nc.scalar.activation(
                        sq[:],
                        o2[:],
                        mybir.ActivationFunctionType.Square,
                        accum_out=s[:],
                    )
                    smax = fpool.tile([P, 1], f32, tag="smax")
                    nc.vector.tensor_scalar_max(smax[:], s[:], 1e-24)
                    rr = fpool.tile([P, 1], f32, tag="rr")
                    if rsqrt_newton:
                        # 1/sqrt via Quake bit-hack seed + 2 Newton steps,
                        # all on DVE (no scalar act-table switches)
                        sh = fpool.tile([P, 1], i32, tag="sh")
                        nc.vector.tensor_scalar(
                            out=sh[:],
                            in0=smax[:].bitcast(i32),
                            scalar1=1,
                            scalar2=None,
                            op0=mybir.AluOpType.logical_shift_right,
                        )
                        y0 = fpool.tile([P, 1], i32, tag="y0")
                        nc.vector.tensor_scalar(
                            out=y0[:],
                            in0=sh[:],
                            scalar1=-1,
                            scalar2=0x5F3759DF + 1,
                            op0=mybir.AluOpType.mult,
                            op1=mybir.AluOpType.add,
                        )
                        y = y0[:].bitcast(f32)
                        for it in range(2):
                            t1 = fpool.tile([P, 1], f32, tag=f"t1_{it}")
                            nc.vector.tensor_tensor(
                                out=t1[:], in0=y, in1=y,
                                op=mybir.AluOpType.mult,
                            )
                            nc.vector.tensor_tensor(
                                out=t1[:], in0=t1[:], in1=smax[:],
                                op=mybir.AluOpType.mult,
                            )
                            nc.vector.tensor_scalar(
                                out=t1[:],
                                in0=t1[:],
                                scalar1=-0.5,
                                scalar2=1.5,
                                op0=mybir.AluOpType.mult,
                                op1=mybir.AluOpType.add,
                            )
                            yn = fpool.tile([P, 1], f32, tag=f"yn_{it}")
                            nc.vector.tensor_tensor(
                                out=yn[:], in0=y, in1=t1[:],
                                op=mybir.AluOpType.mult,
                            )
                            y = yn[:]
                        nc.vector.tensor_copy(rr[:], y)
                    else:
                        # 1/sqrt(s) = exp(-0.5*ln(s))
                        lns = fpool.tile([P, 1], f32, tag="lns")
                        nc.scalar.activation(
                            lns[:], smax[:], mybir.ActivationFunctionType.Ln
                        )
                        nc.scalar.activation(
                            rr[:],
                            lns[:],
                            mybir.ActivationFunctionType.Exp,
                            scale=-0.5,
                        )
                    o3 = fpool.tile([P, HEADS, OUT_CH], f32, tag="o3")
                    nc.vector.tensor_scalar_mul(o3[:], o2[:], rr[:])
                    nc.sync.dma_start(
                        out=out_d[c * P : c * P + pc, :], in_=o3[:pc]
                    )

    nc.compile()
    return nc


# --------------------------------------------------------------------------
# entry point: full inputs in, full output out
# --------------------------------------------------------------------------
def kernel(x, edge_index, W, att_src, att_dst, bias):
    from concourse.bass_utils import run_bass_kernel_spmd

    n_cores = 8
    meta, in_maps = _preprocess(x, edge_index, W, att_src, att_dst, bias, n_cores)
    nc = _build_program(meta, n_cores)
    res = run_bass_kernel_spmd(nc, in_maps, list(range(n_cores)))
    out = np.concatenate([res.results[k]["out"] for k in range(n_cores)], axis=0)
    return out.astype(np.float32)


# revision 8
# speedup vs baseline: 1.3607x; 1.3607x over previous
"""GAT (graph attention) kernel for 8 Trainium2 NeuronCores.

Round-2 (known-good, 1325128 ns): fused edge-replicated dataflow, no
device-side gather.  See kernel.py docstring for the full strategy notes.
"""

import os
import sys

sys.path.insert(0, "/opt/trn_rl_repo")

import numpy as np

HEADS = 4
OUT_CH = 64
NEG_SLOPE = 0.2
P = 128


def _preprocess(x, edge_index, W, att_src, att_dst, bias, n_cores):
    import ml_dtypes

    mask_np = (
        np.float16
        if os.environ.get("GAT_MASK_DTYPE", "f8") == "f16"
        else ml_dtypes.float8_e4m3
    )

    x = np.asarray(x, np.float32)
    N, IN = x.shape
    assert N % n_cores == 0
    npc = N // n_cores
    chunks = (npc + P - 1) // P

    src = np.concatenate(
        [np.asarray(edge_index[0], np.int64), np.arange(N, dtype=np.int64)]
    )
    dst = np.concatenate(
        [np.asarray(edge_index[1], np.int64), np.arange(N, dtype=np.int64)]
    )

    core = dst // npc
    rem = dst - core * npc
    chunk = rem // P
    dstl = (rem - chunk * P).astype(np.int16)

    per_core = []
    for k in range(n_cores):
        sel = np.nonzero(core == k)[0]
        order = np.argsort(chunk[sel], kind="stable")
        sel = sel[order]
        counts = np.bincount(chunk[sel], minlength=chunks)
        per_core.append((src[sel], dstl[sel], counts))

    all_counts = np.stack([pc[2] for pc in per_core])  # [cores, chunks]
    Tch = np.maximum(1, -(-all_counts.max(axis=0) // P))  # [chunks]
    total_slots = int(P * Tch.sum())
    TT = int(Tch.sum())
    tile_off = np.zeros(chunks + 1, np.int64)
    np.cumsum(Tch, out=tile_off[1:])

    xT16 = np.ascontiguousarray(x.T).astype(np.float16)  # [IN, N]

    W4 = np.asarray(W, np.float32).reshape(IN, HEADS, OUT_CH)
    w_src = np.einsum("ihc,hc->ih", W4, np.asarray(att_src, np.float32))
    w_dst = np.einsum("ihc,hc->ih", W4, np.asarray(att_dst, np.float32))
    Wsrc = np.ascontiguousarray(
        np.concatenate([np.asarray(W, np.float32), w_src], axis=1)
    ).astype(np.float16)  # [IN, 260]
    wdst16 = np.ascontiguousarray(w_dst).astype(np.float16)  # [IN, 4]

    d_iota = np.arange(P, dtype=np.int16)
    in_maps = []
    for k in range(n_cores):
        src_k, dstl_k, counts = per_core[k]
        src_slot = np.zeros(total_slots, np.int64)
        dstl_slot = np.full(total_slots, -1, np.int16)
        for c in range(chunks):
            o = int(tile_off[c]) * P
            s0 = int(counts[:c].sum())
            n = int(counts[c])
            src_slot[o : o + n] = src_k[s0 : s0 + n]
            dstl_slot[o : o + n] = dstl_k[s0 : s0 + n]

        dstl_r = dstl_slot.reshape(TT, P)
        m4 = (dstl_r.T[:, :, None] == d_iota[None, None, :]).astype(mask_np)
        mT = (d_iota[:, None, None] == dstl_r[None, :, :]).astype(mask_np)

        in_maps.append(
            {
                "xeT": np.ascontiguousarray(xT16[:, src_slot]),
                "xoT": np.ascontiguousarray(
                    xT16[:, k * npc : (k + 1) * npc]
                ),
                "m4": np.ascontiguousarray(m4.reshape(P, TT * P)),
                "mT": np.ascontiguousarray(mT.reshape(P, TT * P)),
                "Wsrc": Wsrc,
                "wdst": wdst16,
                "bias": np.asarray(bias, np.float32),
            }
        )

    meta = dict(
        N=N, IN=IN, npc=npc, chunks=chunks, Tch=Tch, tile_off=tile_off, TT=TT
    )
    return meta, in_maps


def _build_program(meta, n_cores, debug=False):
    import concourse.bacc as bacc
    import concourse.mybir as mybir
    import concourse.tile as tile

    f32 = mybir.dt.float32
    f16 = mybir.dt.float16
    f8 = mybir.dt.float8e4
    mkdt = f16 if os.environ.get("GAT_MASK_DTYPE", "f8") == "f16" else f8

    N, IN = meta["N"], meta["IN"]
    npc, chunks = meta["npc"], meta["chunks"]
    Tch, tile_off = meta["Tch"], meta["tile_off"]
    TT = meta["TT"]
    AUGS = IN + HEADS  # 260
    KB = IN // P  # contraction blocks (2)
    XB = 8  # edge tiles per xe load

    nc = bacc.Bacc(
        "TRN2", target_bir_lowering=False, debug=debug, num_devices=n_cores
    )

    def mm(out, lhsT, rhs, **kw):
        nc.tensor.matmul(out, lhsT, rhs, **kw)

    xeT_d = nc.dram_tensor("xeT", [IN, TT * P], f16, kind="ExternalInput")
    xoT_d = nc.dram_tensor("xoT", [IN, npc], f16, kind="ExternalInput")
    m4_d = nc.dram_tensor("m4", [P, TT * P], mkdt, kind="ExternalInput")
    mT_d = nc.dram_tensor("mT", [P, TT * P], mkdt, kind="ExternalInput")
    Wsrc_d = nc.dram_tensor("Wsrc", [IN, AUGS], f16, kind="ExternalInput")
    wdst_d = nc.dram_tensor("wdst", [IN, HEADS], f16, kind="ExternalInput")
    bias_d = nc.dram_tensor("bias", [IN], f32, kind="ExternalInput")
    out_d = nc.dram_tensor("out", [npc, IN], f32, kind="ExternalOutput")

    with tile.TileContext(nc) as tc:
        with tc.tile_pool(name="const", bufs=1) as cpool:
            ones_row = cpool.tile([1, P], f32)
            nc.vector.memset(ones_row[:], 1.0)

            bias_row = cpool.tile([1, IN], f32)
            nc.sync.dma_start(out=bias_row[:], in_=bias_d[None, :])
            bias_full = cpool.tile([P, HEADS, OUT_CH], f32)
            with tc.tile_pool(name="cpsum", bufs=1, space="PSUM") as cpsum:
                bias_psum = cpsum.tile([P, HEADS, OUT_CH], f32)
                nc.tensor.matmul(
                    bias_psum[:], ones_row[:], bias_row[:], start=True, stop=True
                )
                nc.vector.tensor_copy(bias_full[:], bias_psum[:])

            Wsrc_sb = cpool.tile([P, KB, AUGS], f16)
            wdst_sb = cpool.tile([P, KB, HEADS], f16)
            for k in range(KB):
                nc.sync.dma_start(
                    out=Wsrc_sb[:, k, :], in_=Wsrc_d[k * P : (k + 1) * P, :]
                )
                nc.sync.dma_start(
                    out=wdst_sb[:, k, :], in_=wdst_d[k * P : (k + 1) * P, :]
                )

            with (
                tc.tile_pool(name="xe", bufs=3) as xepool,
                tc.tile_pool(name="xo", bufs=2) as xopool,
                tc.tile_pool(name="mk", bufs=2) as mkpool,
                tc.tile_pool(name="adst", bufs=2) as adpool,
                tc.tile_pool(name="work", bufs=4) as wpool,
                tc.tile_pool(name="rhs", bufs=4) as rpool,
                tc.tile_pool(name="tail", bufs=2) as fpool,
                tc.tile_pool(name="hpsum", bufs=3, space="PSUM") as hpsum,
                tc.tile_pool(name="opsum", bufs=2, space="PSUM") as opsum,
                tc.tile_pool(name="apsum", bufs=2, space="PSUM") as apsum,
            ):
                for c in range(chunks):
                    Tc = int(Tch[c])
                    toff = int(tile_off[c])
                    pc = min(P, npc - c * P)

                    xo = xopool.tile([P, KB, P], f16, tag="xo")
                    for k in range(KB):
                        nc.scalar.dma_start(
                            out=xo[:, k, :pc],
                            in_=xoT_d[k * P : (k + 1) * P, c * P : c * P + pc],
                        )
                    adp = apsum.tile([P, HEADS], f32)
                    for k in range(KB):
                        mm(
                            adp[:pc, :],
                            xo[:, k, :pc],
                            wdst_sb[:, k, :],
                            start=(k == 0),
                            stop=(k == KB - 1),
                        )
                    adst_sb = adpool.tile([P, HEADS], f16, tag="adst")
                    nc.vector.tensor_copy(adst_sb[:pc, :], adp[:pc, :])

                    m4_sb = mkpool.tile([P, Tc, P], mkdt, tag="m4")
                    nc.sync.dma_start(
                        out=m4_sb[:],
                        in_=m4_d[:, toff * P : (toff + Tc) * P],
                    )
                    mT_sb = mkpool.tile([P, Tc, P], mkdt, tag="mT")
                    nc.sync.dma_start(
                        out=mT_sb[:],
                        in_=mT_d[:, toff * P : (toff + Tc) * P],
                    )

                    out_ps = opsum.tile([P, 4, 65], f32)
                    for t in range(Tc):
                        if t % XB == 0:
                            nxb = min(XB, Tc - t)
                            s0 = (toff + t) * P
                            xe = xepool.tile([P, KB, XB * P], f16, tag="xe")
                            for k in range(KB):
                                eng = nc.sync if (t // XB) % 2 == 0 else nc.scalar
                                eng.dma_start(
                                    out=xe[:, k, : nxb * P],
                                    in_=xeT_d[
                                        k * P : (k + 1) * P, s0 : s0 + nxb * P
                                    ],
                                )
                        xs = (t % XB) * P
                        hp = hpsum.tile([P, AUGS], f32)
                        for k in range(KB):
                            mm(
                                hp[:],
                                xe[:, k, xs : xs + P],
                                Wsrc_sb[:, k, :],
                                start=(k == 0),
                                stop=False,
                            )
                        mm(
                            hp[:, IN : IN + HEADS],
                            mT_sb[:, t, :],
                            adst_sb[:],
                            start=False,
                            stop=True,
                        )
                        e0 = hp[:, IN : IN + HEADS]

                        brhs = rpool.tile([P, 4, 65], f16, tag="brhs")
                        el = wpool.tile([P, HEADS], f16, tag="el")
                        nc.scalar.activation(
                            el[:],
                            e0,
                            mybir.ActivationFunctionType.Prelu,
                            alpha=NEG_SLOPE,
                        )
                        nc.scalar.activation(
                            brhs[:, :, 64],
                            el[:],
                            mybir.ActivationFunctionType.Exp,
                        )
                        nc.vector.tensor_tensor(
                            out=brhs[:, :, 0:64],
                            in0=hp[:, 0:IN].rearrange("p (h c) -> p h c", h=HEADS),
                            in1=brhs[:, :, 64:65].to_broadcast(
                                [P, HEADS, OUT_CH]
                            ),
                            op=mybir.AluOpType.mult,
                        )
                        mm(
                            out_ps[:],
                            m4_sb[:, t, :],
                            brhs[:],
                            start=(t == 0),
                            stop=(t == Tc - 1),
                        )

                    dn = fpool.tile([P, HEADS], f32, tag="dn")
                    nc.vector.tensor_scalar_max(dn[:], out_ps[:, :, 64], 1e-30)
                    rdn = fpool.tile([P, HEADS], f32, tag="rdn")
                    nc.vector.reciprocal(rdn[:], dn[:])
                    o1 = fpool.tile([P, HEADS, OUT_CH], f32, tag="o1")
                    nc.vector.tensor_tensor(
                        out=o1[:],
                        in0=out_ps[:, :, 0:64],
                        in1=rdn[:, :, None].to_broadcast([P, HEADS, OUT_CH]),
                        op=mybir.AluOpType.mult,
                    )
                    nc.vector.tensor_add(o1[:], o1[:], bias_full[:])
                    o2 = fpool.tile([P, HEADS, OUT_CH], f32, tag="o2")
                    nc.scalar.activation(
                        o2[:], o1[:], mybir.ActivationFunctionType.Relu
                    )
                    sq = fpool.tile([P, HEADS, OUT_CH], f16, tag="sq")
                    s = fpool.tile([P, 1], f32, tag="s")
                    nc.scalar.activation(
                        sq[:],
                        o2[:],
                        mybir.ActivationFunctionType.Square,
                        accum_out=s[:],
                    )
                    smax = fpool.tile([P, 1], f32, tag="smax")
                    nc.vector.tensor_scalar_max(smax[:], s[:], 1e-24)
                    lns = fpool.tile([P, 1], f32, tag="lns")
                    nc.scalar.activation(
                        lns[:], smax[:], mybir.ActivationFunctionType.Ln
                    )
                    rr = fpool.tile([P, 1], f32, tag="rr")
                    nc.scalar.activation(
                        rr[:],
                        lns[:],
                        mybir.ActivationFunctionType.Exp,
                        scale=-0.5,
                    )
                    o3 = fpool.tile([P, HEADS, OUT_CH], f32, tag="o3")
                    nc.vector.tensor_scalar_mul(o3[:], o2[:], rr[:])
                    nc.sync.dma_start(
                        out=out_d[c * P : c * P + pc, :], in_=o3[:pc]
                    )

    nc.compile()
    return nc


def kernel(x, edge_index, W, att_src, att_dst, bias):
    from concourse.bass_utils import run_bass_kernel_spmd

    n_cores = 8
    meta, in_maps = _preprocess(x, edge_index, W, att_src, att_dst, bias, n_cores)
    nc = _build_program(meta, n_cores)
    res = run_bass_kernel_spmd(nc, in_maps, list(range(n_cores)))
    out = np.concatenate([res.results[k]["out"] for k in range(n_cores)], axis=0)
    return out.astype(np.float32)
